# revision 34
# baseline (speedup 1.0000x reference)
"""GreedySampler kernel for 8 Trainium2 NeuronCores.

The reference gathers 200 "last token" rows of hidden_states (8
prefill ends + 192 decode slots), computes logits against the
50257x4096 embedding matrix, and takes the argmax over vocab (softmax
and log are monotonic, so argmax(logits) is the answer). The dominant
cost is streaming the embedding matrix: memory-bound.

Plan:
  * Host: compute gather indices from fill_tokens_num /
    num_generation_jobs, gather the 200 rows, transpose to the PE's
    [K, M] layout. Scale embd_weight by 32 (centers its sigma=0.02
    values in fp8-e4m3's normal range), cast both operands to e4m3,
    transpose W to [d, vocab], pad vocab to 50304 (= 8 * 6288, the
    minimal 16-aligned shard), shard over vocab into 8 slices of 6288
    columns (tensor-parallel over vocab), and pre-swizzle each shard
    into the exact per-tile SBUF layout so every W DMA is one fully
    contiguous block.
  * Device (SPMD, 8 cores): stream the 25.8MB W-shard once as 26
    vocab tiles (24x256 + 128 + 16; the small tail tiles keep the
    post-stream dependency chain short). W chunks are the stationary
    matmul operand, the 200 job rows the moving operand, with fp8
    DoubleRowSwInterleave packing K=256 per pass (weights pre
    interleaved on host so the PE weight load reads SBUF
    contiguously, HW-verified layout: A/B pairs per column, columns
    reversed within each 128-col subtile). Accumulation is fp32 in
    PSUM; logits go out as fp8 in a [128, 50, 200] layout whose per
    partition runs are >=512B contiguous (no small-descriptor DMA
    penalty), grouped 8 subtiles per output DMA with the last group
    split per-subtile across both DGE queues. W-tile loads ride
    nc.sync (HWDGE, shortest init) 5-deep-buffered so the DMA engines
    never idle; hst and outputs ride nc.scalar so the two streams
    don't share a sequencer; each engine's first DMA is hoisted above
    the Tile preamble barrier so the first transfer starts ~0.8us
    earlier.
  * Host: per-row global max over the gathered approximate logits;
    every column within DELTA of the max (fp8 logit error measured at
    <=0.28 in unscaled units; DELTA=2.0 is a ~7x margin on the max
    observed error, ~30 sigma) is rescored exactly in float64 against
    the original fp32 weights. The argmax of exact scores equals the
    fp32 reference argmax — quantization only shortlists candidates,
    it never decides the winner.

Notes:
  * This walrus build rejects instructions carrying more than one sync
    wait, so after Tile scheduling we split excess waits onto nop
    instructions inserted just before the offender on the same engine
    queue (in-order execution keeps the semantics identical).
  * DoubleRow(SwInterleave) AP contract: lhsT [128, 2, M] (free =
    2M), rhs [128, 2, N] (free = 2N), out [M, N]; both operands here
    use d = kk*256 + t*128 + p so the packing is consistent. For
    SwInterleave the lhsT AP has t stride 1 / m stride 2 over the
    pre-interleaved buffer (verified against DoubleRow on hardware).
"""

import math

import numpy as np
import ml_dtypes

import concourse.bass as bass
import concourse.mybir as mybir
import concourse.tile as tile
from concourse.vector_clock import ScopedClock
from concourse.bass_utils import run_bass_kernel_spmd

P = 128
N_CORES = 8
KK = 16  # 4096 / 256 DoubleRow K-chunks
VS = 6288  # per-core vocab shard width (50304 / 8)
TILE_WIDTHS = [256] * 24 + [128, 16]  # sum = 6288
NSUB_TOT = 50  # 24*2 + 1 + 1 subtiles of (up to) 128 vocab cols
W_SCALE = 32.0
DELTA = 2.0 * W_SCALE  # candidate margin in scaled-logit units

FP8 = mybir.dt.float8e4
F32 = mybir.dt.float32

_drain_patched = False


def _patch_tile_drain():
    """Split the tail Drain's sync waits (>1 rejected by this walrus)."""
    global _drain_patched
    if _drain_patched:
        return

    def _drain_and_barrier(self, tick_clock, wait_clock):
        nc = self.nc
        drain_inst = nc.sync.drain()
        wait_clock.add_sem_waits(
            drain_inst.ins, ScopedClock({None: tick_clock.global_clock})
        )
        si = drain_inst.ins.sync_info
        if si is not None and si.on_wait and len(si.on_wait) > 1:
            extra = list(si.on_wait[1:])
            del si.on_wait[1:]
            # Wait on the semaphores that fire last (the final DMAs'
            # lanes) last, so no 50ns wait-decodes trail the final
            # completion.
            late = getattr(nc, "_drain_late_sem_prefixes", ())
            extra.sort(
                key=lambda w: next(
                    (k + 1 for k, p in enumerate(late)
                     if w.ant_name.startswith(p)),
                    0,
                )
            )
            name2sem = {
                getattr(s, "name", None): s
                for s in self.sems.allocated().values()
            }
            for w in extra:
                nc.sync.wait_ge(name2sem[w.ant_name], w.wait_value)
        nc.all_engine_barrier()
        popped = nc._tile_sem_poison_stack.pop()
        assert popped is self._sem_poison
        nc.clear_and_free_semaphores(list(self.sems.allocated().values()))
        nc.all_engine_barrier()

    tile.TileContext._drain_and_barrier = _drain_and_barrier
    _drain_patched = True


def _split_excess_waits(nc, limit=1):
    """Move all but `limit` sync waits of every instruction onto nops
    inserted immediately before it on the same engine queue."""
    fn = nc.m.functions[0]
    for bb in fn.blocks:
        if not any(
            getattr(i, "sync_info", None) is not None
            and i.sync_info.on_wait
            and len(i.sync_info.on_wait) > limit
            for i in bb.instructions
        ):
            continue
        cur = nc.cur_bb.bb if hasattr(nc.cur_bb, "bb") else nc.cur_bb
        new_insts = []
        for inst in bb.instructions:
            si = getattr(inst, "sync_info", None)
            if si is not None and si.on_wait and len(si.on_wait) > limit:
                extra = list(si.on_wait[:-limit])
                del si.on_wait[: len(si.on_wait) - limit]
                for w in extra:
                    nop = nc.engines[inst.engine].nop(nofuse=True).ins
                    popped = cur.instructions.pop()  # nop() self-appended
                    assert popped is nop
                    nop.sync_info = mybir.SyncInfo(on_wait=[w], on_update=[])
                    new_insts.append(nop)
            new_insts.append(inst)
        bb.instructions[:] = new_insts
    return nc


def max_waits(nc):
    worst = 0
    for bb in nc.m.functions[0].blocks:
        for inst in bb.instructions:
            si = getattr(inst, "sync_info", None)
            if si is not None and si.on_wait:
                worst = max(worst, len(si.on_wait))
    return worst


def build_nc(
    J,
    widths=None,
    w_bufs=5,
    out_group=8,
    ps_bufs=8,
    out_bufs=4,
    out_groups=None,
    kk_split=1,
):
    """One core: approximate logits for its 6288-column vocab shard.

    lt[p, s, j] = (hs[j, :] @ wt[:, s*128 + p]).T in fp8, fp32
    accumulation. Vocab W tiles stream in HWDGE multi-buffered; job
    rows are the moving matmul operand. Outputs are DMAed in groups
    of `out_group` 128-col subtiles (contiguous >=512B runs, no
    small-descriptor penalty); the final group is the short 128+16
    tail so the post-stream dependency chain stays small.
    """
    _patch_tile_drain()
    if widths is None:
        widths = TILE_WIDTHS
    assert sum(widths) == VS and all(
        w % P == 0 for w in widths[:-1]
    ) and widths[-1] % 16 == 0
    if out_groups is None:
        out_groups = []
        left = NSUB_TOT
        while left > 0:
            g = min(out_group, left)
            out_groups.append(g)
            left -= g
    assert sum(out_groups) == NSUB_TOT
    group_of = []  # group index per subtile
    for gi, g in enumerate(out_groups):
        group_of += [gi] * g
    # HWDGE lane of the last two DMAs issued (the split tail outputs):
    # their completion sems fire last, so the drain waits on them last.
    n_hwdge = len(widths) * kk_split + 1 + (len(out_groups) - 1) + out_groups[-1]

    nc = bass.Bass()
    hst = nc.dram_tensor("hst", [P, KK, 2, J], FP8, kind="ExternalInput")
    # One flat W tensor (pre-swizzled per-tile blocks, concatenated):
    # fewer kernel arguments keeps per-dispatch overhead close to the
    # tiny-kernel reference used by the differential timer. Within each
    # 128-col subtile the two K-rows are software-interleaved pairwise
    # with columns reversed (DoubleRowSwInterleave layout), so the PE
    # weight load reads SBUF contiguously instead of the DoubleRow
    # hardware interleave's strided pattern.
    wt = nc.dram_tensor("wt", [VS * 4096], FP8, kind="ExternalInput")
    wts = []
    off = 0
    for w in widths:
        n = P * KK * 2 * w
        wts.append(
            wt[off : off + n].rearrange(
                "(p kk q) -> p kk q", p=P, kk=KK, q=2 * w
            )
        )
        off += n
    lt = nc.dram_tensor("lt", [P, NSUB_TOT, J], FP8, kind="ExternalOutput")
    nc._drain_late_sem_prefixes = (
        f"DMAHW{(n_hwdge - 2) % 8}_",
        f"DMAHW{(n_hwdge - 1) % 8}_",
    )

    with tile.TileContext(nc) as tc:
        with (
            tc.tile_pool(name="hs", bufs=1) as hs_pool,
            tc.tile_pool(name="w", bufs=w_bufs) as w_pool,
            tc.tile_pool(name="out", bufs=out_bufs) as out_pool,
            tc.tile_pool(name="ps", bufs=ps_bufs, space=bass.MemorySpace.PSUM) as ps_pool,
        ):
            # First DMA on sync/HWDGE (shortest init) keeps the DMA
            # engines busy from ~1.3us; hst rides gpsimd concurrently
            # and slots in behind tile 0's transfer.
            hst_sb = None
            s_global = 0  # global 128-col subtile index
            ot = None
            ot_s0 = 0
            ot_n = 0
            for i, w in enumerate(widths):
                w_sb = w_pool.tile([P, KK, 2 * w], FP8, name=f"w_sb{w}")
                if kk_split > 1:
                    # Split the tile load along KK so the accumulation
                    # chain starts while the rest of the tile streams.
                    step = KK // kk_split
                    for h in range(kk_split):
                        nc.sync.dma_start(
                            w_sb[:, h * step : (h + 1) * step],
                            wts[i][:, h * step : (h + 1) * step],
                        )
                else:
                    nc.sync.dma_start(w_sb[:], wts[i][:])
                if hst_sb is None:
                    hst_sb = hs_pool.tile([P, KK, 2, J], FP8)
                    nc.scalar.dma_start(hst_sb[:], hst[:])

                for sub in range(math.ceil(w / P)):
                    m = min(P, w - sub * P)
                    if ot is None:
                        ot_n = out_groups[group_of[s_global]]
                        ot = out_pool.tile(
                            [P, max(out_groups), J], FP8, name="ot"
                        )
                        ot_s0 = s_global
                    ps = ps_pool.tile([P, J], F32, name="ps")
                    for kk in range(KK):
                        if m == P:
                            # Full subtile: software-interleaved layout,
                            # contiguous weight load.
                            lhsT = w_sb[
                                :, kk, 2 * sub * P : 2 * (sub * P + m)
                            ].rearrange("p (mm t) -> p t mm", t=2)
                            mode = mybir.MatmulPerfMode.DoubleRowSwInterleave
                        else:
                            # Narrow tail subtile: walrus rejects
                            # SwInterleave below full column count
                            # (s3_lw_valid_num_active_cols); use plain
                            # DoubleRow on the non-interleaved layout.
                            lhsT = w_sb[
                                :, kk, 2 * sub * P : 2 * sub * P + 2 * m
                            ].rearrange("p (t mm) -> p t mm", t=2)
                            mode = mybir.MatmulPerfMode.DoubleRow
                        nc.tensor.matmul(
                            ps[:m, :J],
                            lhsT,
                            hst_sb[:, kk, :, :],
                            start=(kk == 0),
                            stop=(kk == KK - 1),
                            perf_mode=mode,
                        )
                    nc.vector.tensor_copy(
                        ot[:m, s_global - ot_s0, :], ps[:m, :J]
                    )
                    s_global += 1
                    if s_global - ot_s0 == ot_n:
                        if s_global == NSUB_TOT:
                            # Last group: one DMA per subtile, the final one
                            # on the otherwise-idle SP queue, so each output
                            # leaves as soon as its copy lands instead of
                            # serializing behind the whole group.
                            for g in range(ot_n):
                                eng = nc.sync if g == ot_n - 1 else nc.scalar
                                eng.dma_start(
                                    lt[:, ot_s0 + g : ot_s0 + g + 1, :],
                                    ot[:, g : g + 1, :],
                                )
                        else:
                            nc.scalar.dma_start(
                                lt[:, ot_s0 : ot_s0 + ot_n, :], ot[:, :ot_n, :]
                            )
                        ot = None

    _hoist_first_dmas(nc)
    _split_excess_waits(nc, limit=1)
    return nc


def _hoist_first_dmas(nc):
    """Move each engine's first wait-free DMACopy from the body block to
    just before that engine's preamble-barrier Drain, so the transfer
    runs during the barrier instead of after it. The DMAs have no sync
    waits, their target SBUF tiles are untouched by any preamble
    instruction (the only preamble SBUF writes are Pool's SWDGE-ring
    carveout), and their completion sems fire microseconds after the
    sem init completes, so ordering is preserved on hardware."""
    fn = nc.m.functions[0]
    pre, body = fn.blocks[0], fn.blocks[1]
    for eng in (mybir.EngineType.SP, mybir.EngineType.Activation):
        cand = None
        for inst in body.instructions:
            if inst.engine != eng:
                continue
            si = getattr(inst, "sync_info", None)
            if inst.opcode == "DMACopy" and not (si and si.on_wait):
                cand = inst
            break  # only consider the engine's first body instruction
        if cand is None:
            continue
        body.instructions.remove(cand)
        drain_pos = next(
            k
            for k, inst in enumerate(pre.instructions)
            if inst.engine == eng and inst.opcode == "Drain"
        )
        pre.instructions.insert(drain_pos, cand)


def _job_indices(fill_tokens_num, num_generation_jobs):
    fill = np.asarray(fill_tokens_num, dtype=np.int64)
    fill_last = np.cumsum(fill) - 1
    total_fill = int(fill.sum())
    gen = total_fill + np.arange(int(num_generation_jobs), dtype=np.int64)
    return np.concatenate([fill_last, gen])


def _swizzle(mat_t):
    """[D, N] -> [P, KK, 2, N] with d = kk*256 + t*128 + p."""
    D, N = mat_t.shape
    return np.ascontiguousarray(
        mat_t.reshape(D // 256, 2, P, N).transpose(2, 0, 1, 3)
    )


def _swi_interleave(T):
    """[P, KK, 2, w] -> [P, KK, 2w]: per full 128-col subtile, software
    DoubleRow interleave (A/B pairs per column, columns reversed):
    per (p, kk): A_{M-1} B_{M-1} A_{M-2} B_{M-2} ... A_0 B_0.
    Narrow tail subtiles (M < 128) stay in plain [t][m] halves layout
    for the DoubleRow fallback the device uses there."""
    Pd, KKd, _, w = T.shape
    out = np.empty((Pd, KKd, 2 * w), T.dtype)
    for s0 in range(0, w, P):
        M = min(P, w - s0)
        sub = T[:, :, :, s0 : s0 + M]
        if M == P:
            rev = sub[:, :, :, ::-1]
            out[:, :, 2 * s0 : 2 * (s0 + M) : 2] = rev[:, :, 0]
            out[:, :, 2 * s0 + 1 : 2 * (s0 + M) : 2] = rev[:, :, 1]
        else:
            out[:, :, 2 * s0 : 2 * s0 + M] = sub[:, :, 0]
            out[:, :, 2 * s0 + M : 2 * s0 + 2 * M] = sub[:, :, 1]
    return out


def shard_logits(res_lt, J):
    """[128, 50, J] fp8 device output -> [VS, J] float32 (scaled)."""
    return (
        np.asarray(res_lt)
        .astype(np.float32)
        .transpose(1, 0, 2)
        .reshape(NSUB_TOT * P, J)[:VS]
    )


def kernel(hidden_states, embd_weight, fill_tokens_num, num_generation_jobs):
    hs = np.asarray(hidden_states, dtype=np.float32)
    W = np.asarray(embd_weight, dtype=np.float32)
    V, D = W.shape

    idx = _job_indices(fill_tokens_num, num_generation_jobs)
    J = idx.size

    hs_sel = hs[idx]  # [J, D] f32, kept for the exact rescore
    hst_host = _swizzle(hs_sel.T.astype(ml_dtypes.float8_e4m3))

    V_pad = VS * N_CORES
    Wq = (W * W_SCALE).astype(ml_dtypes.float8_e4m3)
    WT_pad = np.zeros((D, V_pad), dtype=ml_dtypes.float8_e4m3)
    WT_pad[:, :V] = Wq.T

    in_maps = []
    for c in range(N_CORES):
        shard_sw = _swizzle(WT_pad[:, c * VS : (c + 1) * VS])
        blocks = []
        off = 0
        for w in TILE_WIDTHS:
            blocks.append(
                _swi_interleave(shard_sw[:, :, :, off : off + w]).reshape(-1)
            )
            off += w
        in_maps.append(
            {"hst": hst_host, "wt": np.ascontiguousarray(np.concatenate(blocks))}
        )

    nc = build_nc(J)
    kernel.last_nc = nc
    kernel.last_in_maps = in_maps
    res = run_bass_kernel_spmd(nc, in_maps, core_ids=list(range(N_CORES)))
    kernel.last_results = res

    # [J, V_pad] -> crop pad; values are scaled by W_SCALE (irrelevant
    # for ranking, DELTA is in the same scaled units)
    logits = np.concatenate(
        [shard_logits(res.results[c]["lt"], J) for c in range(N_CORES)],
        axis=0,
    ).T[:, :V]
    # Device e4m3fn values above 240 decode as inf/NaN under ml_dtypes'
    # IEEE e4m3. Quantization is monotone, so the true argmax always
    # ties the row max and stays a candidate; map NaN to +inf so such
    # columns are candidates (rescoring decides) rather than poisoning
    # the row max.
    logits = np.where(np.isnan(logits), np.inf, logits)

    # Columns within DELTA of each row's max, rescored exactly in f64.
    m = logits.max(axis=1, keepdims=True)
    rows, cols = np.nonzero(logits >= m - DELTA)
    exact = np.einsum(
        "ij,ij->i", hs_sel[rows].astype(np.float64), W[cols].astype(np.float64)
    )
    ids = np.zeros(J, dtype=np.int64)
    best = np.full(J, -np.inf)
    for r, c, s in zip(rows, cols, exact):
        if s > best[r]:
            best[r] = s
            ids[r] = c
    return ids.astype(np.int32)


# revision 35
# speedup vs baseline: 1.0015x; 1.0015x over previous
"""GreedySampler kernel for 8 Trainium2 NeuronCores.

The reference gathers 200 "last token" rows of hidden_states (8
prefill ends + 192 decode slots), computes logits against the
50257x4096 embedding matrix, and takes the argmax over vocab (softmax
and log are monotonic, so argmax(logits) is the answer). The dominant
cost is streaming the embedding matrix: memory-bound.

Plan:
  * Host: compute gather indices from fill_tokens_num /
    num_generation_jobs, gather the 200 rows, transpose to the PE's
    [K, M] layout. Scale embd_weight by 32 (centers its sigma=0.02
    values in fp8-e4m3's normal range), cast both operands to e4m3,
    transpose W to [d, vocab], pad vocab to 50304 (= 8 * 6288, the
    minimal 16-aligned shard), shard over vocab into 8 slices of 6288
    columns (tensor-parallel over vocab), and pre-swizzle each shard
    into the exact per-tile SBUF layout so every W DMA is one fully
    contiguous block.
  * Device (SPMD, 8 cores): stream the 25.8MB W-shard once as 26
    vocab tiles (24x256 + 128 + 16; the small tail tiles keep the
    post-stream dependency chain short). W chunks are the stationary
    matmul operand, the 200 job rows the moving operand, with fp8
    DoubleRowSwInterleave packing K=256 per pass (weights pre
    interleaved on host so the PE weight load reads SBUF
    contiguously, HW-verified layout: A/B pairs per column, columns
    reversed within each 128-col subtile). Accumulation is fp32 in
    PSUM; logits go out as fp8 in a [128, 50, 200] layout whose per
    partition runs are >=512B contiguous (no small-descriptor DMA
    penalty), grouped 8 subtiles per output DMA with the last group
    split per-subtile across both DGE queues. W-tile loads ride
    nc.sync (HWDGE, shortest init) 5-deep-buffered so the DMA engines
    never idle; hst and outputs ride nc.scalar so the two streams
    don't share a sequencer; each engine's first DMA is hoisted above
    the Tile preamble barrier so the first transfer starts ~0.8us
    earlier.
  * Host: per-row global max over the gathered approximate logits;
    every column within DELTA of the max (fp8 logit error measured at
    <=0.28 in unscaled units; DELTA=2.0 is a ~7x margin on the max
    observed error, ~30 sigma) is rescored exactly in float64 against
    the original fp32 weights. The argmax of exact scores equals the
    fp32 reference argmax — quantization only shortlists candidates,
    it never decides the winner.

Notes:
  * This walrus build rejects instructions carrying more than one sync
    wait, so after Tile scheduling we split excess waits onto nop
    instructions inserted just before the offender on the same engine
    queue (in-order execution keeps the semantics identical).
  * DoubleRow(SwInterleave) AP contract: lhsT [128, 2, M] (free =
    2M), rhs [128, 2, N] (free = 2N), out [M, N]; both operands here
    use d = kk*256 + t*128 + p so the packing is consistent. For
    SwInterleave the lhsT AP has t stride 1 / m stride 2 over the
    pre-interleaved buffer (verified against DoubleRow on hardware).
"""

import math

import numpy as np
import ml_dtypes

import concourse.bass as bass
import concourse.mybir as mybir
import concourse.tile as tile
from concourse.vector_clock import ScopedClock
from concourse.bass_utils import run_bass_kernel_spmd

P = 128
N_CORES = 8
KK = 16  # 4096 / 256 DoubleRow K-chunks
VS = 6288  # per-core vocab shard width (50304 / 8)
TILE_WIDTHS = [256] * 24 + [128, 16]  # sum = 6288
NSUB_TOT = 50  # 24*2 + 1 + 1 subtiles of (up to) 128 vocab cols
W_SCALE = 32.0
DELTA = 2.0 * W_SCALE  # candidate margin in scaled-logit units

FP8 = mybir.dt.float8e4
F32 = mybir.dt.float32

_drain_patched = False


def _patch_tile_drain():
    """Split the tail Drain's sync waits (>1 rejected by this walrus)."""
    global _drain_patched
    if _drain_patched:
        return

    def _drain_and_barrier(self, tick_clock, wait_clock):
        nc = self.nc
        drain_inst = nc.sync.drain()
        wait_clock.add_sem_waits(
            drain_inst.ins, ScopedClock({None: tick_clock.global_clock})
        )
        si = drain_inst.ins.sync_info
        if si is not None and si.on_wait and len(si.on_wait) > 1:
            extra = list(si.on_wait[1:])
            del si.on_wait[1:]
            # Wait on the semaphores that fire last (the final DMAs'
            # lanes) last, so no 50ns wait-decodes trail the final
            # completion.
            late = getattr(nc, "_drain_late_sem_prefixes", ())
            extra.sort(
                key=lambda w: next(
                    (k + 1 for k, p in enumerate(late)
                     if w.ant_name.startswith(p)),
                    0,
                )
            )
            name2sem = {
                getattr(s, "name", None): s
                for s in self.sems.allocated().values()
            }
            for w in extra:
                nc.sync.wait_ge(name2sem[w.ant_name], w.wait_value)
        nc.all_engine_barrier()
        popped = nc._tile_sem_poison_stack.pop()
        assert popped is self._sem_poison
        nc.clear_and_free_semaphores(list(self.sems.allocated().values()))
        nc.all_engine_barrier()

    tile.TileContext._drain_and_barrier = _drain_and_barrier
    _drain_patched = True


def _split_excess_waits(nc, limit=1):
    """Move all but `limit` sync waits of every instruction onto nops
    inserted immediately before it on the same engine queue."""
    fn = nc.m.functions[0]
    for bb in fn.blocks:
        if not any(
            getattr(i, "sync_info", None) is not None
            and i.sync_info.on_wait
            and len(i.sync_info.on_wait) > limit
            for i in bb.instructions
        ):
            continue
        cur = nc.cur_bb.bb if hasattr(nc.cur_bb, "bb") else nc.cur_bb
        new_insts = []
        for inst in bb.instructions:
            si = getattr(inst, "sync_info", None)
            if si is not None and si.on_wait and len(si.on_wait) > limit:
                extra = list(si.on_wait[:-limit])
                del si.on_wait[: len(si.on_wait) - limit]
                for w in extra:
                    nop = nc.engines[inst.engine].nop(nofuse=True).ins
                    popped = cur.instructions.pop()  # nop() self-appended
                    assert popped is nop
                    nop.sync_info = mybir.SyncInfo(on_wait=[w], on_update=[])
                    new_insts.append(nop)
            new_insts.append(inst)
        bb.instructions[:] = new_insts
    return nc


def max_waits(nc):
    worst = 0
    for bb in nc.m.functions[0].blocks:
        for inst in bb.instructions:
            si = getattr(inst, "sync_info", None)
            if si is not None and si.on_wait:
                worst = max(worst, len(si.on_wait))
    return worst


def build_nc(
    J,
    widths=None,
    w_bufs=5,
    out_group=8,
    ps_bufs=8,
    out_bufs=4,
    out_groups=None,
    kk_split=1,
):
    """One core: approximate logits for its 6288-column vocab shard.

    lt[p, s, j] = (hs[j, :] @ wt[:, s*128 + p]).T in fp8, fp32
    accumulation. Vocab W tiles stream in HWDGE multi-buffered; job
    rows are the moving matmul operand. Outputs are DMAed in groups
    of `out_group` 128-col subtiles (contiguous >=512B runs, no
    small-descriptor penalty); the final group is the short 128+16
    tail so the post-stream dependency chain stays small.
    """
    _patch_tile_drain()
    if widths is None:
        widths = TILE_WIDTHS
    assert sum(widths) == VS and all(
        w % P == 0 for w in widths[:-1]
    ) and widths[-1] % 16 == 0
    if out_groups is None:
        out_groups = []
        left = NSUB_TOT
        while left > 0:
            g = min(out_group, left)
            out_groups.append(g)
            left -= g
    assert sum(out_groups) == NSUB_TOT
    group_of = []  # group index per subtile
    for gi, g in enumerate(out_groups):
        group_of += [gi] * g
    # HWDGE lane of the last two DMAs issued (the split tail outputs):
    # their completion sems fire last, so the drain waits on them last.
    n_hwdge = len(widths) * kk_split + 1 + (len(out_groups) - 1) + out_groups[-1]

    nc = bass.Bass()
    hst = nc.dram_tensor("hst", [P, KK, 2, J], FP8, kind="ExternalInput")
    # One flat W tensor (pre-swizzled per-tile blocks, concatenated):
    # fewer kernel arguments keeps per-dispatch overhead close to the
    # tiny-kernel reference used by the differential timer. Within each
    # 128-col subtile the two K-rows are software-interleaved pairwise
    # with columns reversed (DoubleRowSwInterleave layout), so the PE
    # weight load reads SBUF contiguously instead of the DoubleRow
    # hardware interleave's strided pattern.
    wt = nc.dram_tensor("wt", [VS * 4096], FP8, kind="ExternalInput")
    wts = []
    off = 0
    for w in widths:
        n = P * KK * 2 * w
        wts.append(
            wt[off : off + n].rearrange(
                "(p kk q) -> p kk q", p=P, kk=KK, q=2 * w
            )
        )
        off += n
    lt = nc.dram_tensor("lt", [P, NSUB_TOT, J], FP8, kind="ExternalOutput")
    nc._drain_late_sem_prefixes = (
        f"DMAHW{(n_hwdge - 2) % 8}_",
        f"DMAHW{(n_hwdge - 1) % 8}_",
    )

    with tile.TileContext(nc) as tc:
        with (
            tc.tile_pool(name="hs", bufs=1) as hs_pool,
            tc.tile_pool(name="w", bufs=w_bufs) as w_pool,
            tc.tile_pool(name="out", bufs=out_bufs) as out_pool,
            tc.tile_pool(name="ps", bufs=ps_bufs, space=bass.MemorySpace.PSUM) as ps_pool,
        ):
            # First DMA on sync/HWDGE (shortest init) keeps the DMA
            # engines busy from ~1.3us; hst rides gpsimd concurrently
            # and slots in behind tile 0's transfer.
            hst_sb = None
            s_global = 0  # global 128-col subtile index
            ot = None
            ot_s0 = 0
            ot_n = 0
            for i, w in enumerate(widths):
                w_sb = w_pool.tile([P, KK, 2 * w], FP8, name=f"w_sb{w}")
                if kk_split > 1:
                    # Split the tile load along KK so the accumulation
                    # chain starts while the rest of the tile streams.
                    step = KK // kk_split
                    for h in range(kk_split):
                        nc.sync.dma_start(
                            w_sb[:, h * step : (h + 1) * step],
                            wts[i][:, h * step : (h + 1) * step],
                        )
                else:
                    nc.sync.dma_start(w_sb[:], wts[i][:])
                if hst_sb is None:
                    hst_sb = hs_pool.tile([P, KK, 2, J], FP8)
                    nc.scalar.dma_start(hst_sb[:], hst[:])

                for sub in range(math.ceil(w / P)):
                    m = min(P, w - sub * P)
                    if ot is None:
                        ot_n = out_groups[group_of[s_global]]
                        ot = out_pool.tile(
                            [P, max(out_groups), J], FP8, name="ot"
                        )
                        ot_s0 = s_global
                    ps = ps_pool.tile([P, J], F32, name="ps")
                    for kk in range(KK):
                        if m == P:
                            # Full subtile: software-interleaved layout,
                            # contiguous weight load.
                            lhsT = w_sb[
                                :, kk, 2 * sub * P : 2 * (sub * P + m)
                            ].rearrange("p (mm t) -> p t mm", t=2)
                            mode = mybir.MatmulPerfMode.DoubleRowSwInterleave
                        else:
                            # Narrow tail subtile: walrus rejects
                            # SwInterleave below full column count
                            # (s3_lw_valid_num_active_cols); use plain
                            # DoubleRow on the non-interleaved layout.
                            lhsT = w_sb[
                                :, kk, 2 * sub * P : 2 * sub * P + 2 * m
                            ].rearrange("p (t mm) -> p t mm", t=2)
                            mode = mybir.MatmulPerfMode.DoubleRow
                        nc.tensor.matmul(
                            ps[:m, :J],
                            lhsT,
                            hst_sb[:, kk, :, :],
                            start=(kk == 0),
                            stop=(kk == KK - 1),
                            perf_mode=mode,
                        )
                    nc.vector.tensor_copy(
                        ot[:m, s_global - ot_s0, :], ps[:m, :J]
                    )
                    s_global += 1
                    if s_global - ot_s0 == ot_n:
                        if s_global == NSUB_TOT:
                            # Last group: one DMA per subtile, the final one
                            # on the otherwise-idle SP queue, so each output
                            # leaves as soon as its copy lands instead of
                            # serializing behind the whole group. The narrow
                            # tail subtile only transfers its valid
                            # partitions (the host crops the rest anyway).
                            for g in range(ot_n):
                                eng = nc.sync if g == ot_n - 1 else nc.scalar
                                pm = (
                                    P
                                    if ot_s0 + g < NSUB_TOT - 1
                                    else VS - (NSUB_TOT - 1) * P
                                )
                                eng.dma_start(
                                    lt[:pm, ot_s0 + g : ot_s0 + g + 1, :],
                                    ot[:pm, g : g + 1, :],
                                )
                        else:
                            nc.scalar.dma_start(
                                lt[:, ot_s0 : ot_s0 + ot_n, :], ot[:, :ot_n, :]
                            )
                        ot = None

    _hoist_first_dmas(nc)
    _split_excess_waits(nc, limit=1)
    return nc


def _hoist_first_dmas(nc):
    """Move each engine's first wait-free DMACopy from the body block to
    just before that engine's preamble-barrier Drain, so the transfer
    runs during the barrier instead of after it. The DMAs have no sync
    waits, their target SBUF tiles are untouched by any preamble
    instruction (the only preamble SBUF writes are Pool's SWDGE-ring
    carveout), and their completion sems fire microseconds after the
    sem init completes, so ordering is preserved on hardware."""
    fn = nc.m.functions[0]
    pre, body = fn.blocks[0], fn.blocks[1]
    for eng in (mybir.EngineType.SP, mybir.EngineType.Activation):
        cand = None
        for inst in body.instructions:
            if inst.engine != eng:
                continue
            si = getattr(inst, "sync_info", None)
            if inst.opcode == "DMACopy" and not (si and si.on_wait):
                cand = inst
            break  # only consider the engine's first body instruction
        if cand is None:
            continue
        body.instructions.remove(cand)
        drain_pos = next(
            k
            for k, inst in enumerate(pre.instructions)
            if inst.engine == eng and inst.opcode == "Drain"
        )
        pre.instructions.insert(drain_pos, cand)


def _job_indices(fill_tokens_num, num_generation_jobs):
    fill = np.asarray(fill_tokens_num, dtype=np.int64)
    fill_last = np.cumsum(fill) - 1
    total_fill = int(fill.sum())
    gen = total_fill + np.arange(int(num_generation_jobs), dtype=np.int64)
    return np.concatenate([fill_last, gen])


def _swizzle(mat_t):
    """[D, N] -> [P, KK, 2, N] with d = kk*256 + t*128 + p."""
    D, N = mat_t.shape
    return np.ascontiguousarray(
        mat_t.reshape(D // 256, 2, P, N).transpose(2, 0, 1, 3)
    )


def _swi_interleave(T):
    """[P, KK, 2, w] -> [P, KK, 2w]: per full 128-col subtile, software
    DoubleRow interleave (A/B pairs per column, columns reversed):
    per (p, kk): A_{M-1} B_{M-1} A_{M-2} B_{M-2} ... A_0 B_0.
    Narrow tail subtiles (M < 128) stay in plain [t][m] halves layout
    for the DoubleRow fallback the device uses there."""
    Pd, KKd, _, w = T.shape
    out = np.empty((Pd, KKd, 2 * w), T.dtype)
    for s0 in range(0, w, P):
        M = min(P, w - s0)
        sub = T[:, :, :, s0 : s0 + M]
        if M == P:
            rev = sub[:, :, :, ::-1]
            out[:, :, 2 * s0 : 2 * (s0 + M) : 2] = rev[:, :, 0]
            out[:, :, 2 * s0 + 1 : 2 * (s0 + M) : 2] = rev[:, :, 1]
        else:
            out[:, :, 2 * s0 : 2 * s0 + M] = sub[:, :, 0]
            out[:, :, 2 * s0 + M : 2 * s0 + 2 * M] = sub[:, :, 1]
    return out


def shard_logits(res_lt, J):
    """[128, 50, J] fp8 device output -> [VS, J] float32 (scaled)."""
    return (
        np.asarray(res_lt)
        .astype(np.float32)
        .transpose(1, 0, 2)
        .reshape(NSUB_TOT * P, J)[:VS]
    )


def kernel(hidden_states, embd_weight, fill_tokens_num, num_generation_jobs):
    hs = np.asarray(hidden_states, dtype=np.float32)
    W = np.asarray(embd_weight, dtype=np.float32)
    V, D = W.shape

    idx = _job_indices(fill_tokens_num, num_generation_jobs)
    J = idx.size

    hs_sel = hs[idx]  # [J, D] f32, kept for the exact rescore
    hst_host = _swizzle(hs_sel.T.astype(ml_dtypes.float8_e4m3))

    V_pad = VS * N_CORES
    Wq = (W * W_SCALE).astype(ml_dtypes.float8_e4m3)
    WT_pad = np.zeros((D, V_pad), dtype=ml_dtypes.float8_e4m3)
    WT_pad[:, :V] = Wq.T

    in_maps = []
    for c in range(N_CORES):
        shard_sw = _swizzle(WT_pad[:, c * VS : (c + 1) * VS])
        blocks = []
        off = 0
        for w in TILE_WIDTHS:
            blocks.append(
                _swi_interleave(shard_sw[:, :, :, off : off + w]).reshape(-1)
            )
            off += w
        in_maps.append(
            {"hst": hst_host, "wt": np.ascontiguousarray(np.concatenate(blocks))}
        )

    nc = build_nc(J)
    kernel.last_nc = nc
    kernel.last_in_maps = in_maps
    res = run_bass_kernel_spmd(nc, in_maps, core_ids=list(range(N_CORES)))
    kernel.last_results = res

    # [J, V_pad] -> crop pad; values are scaled by W_SCALE (irrelevant
    # for ranking, DELTA is in the same scaled units)
    logits = np.concatenate(
        [shard_logits(res.results[c]["lt"], J) for c in range(N_CORES)],
        axis=0,
    ).T[:, :V]
    # Device e4m3fn values above 240 decode as inf/NaN under ml_dtypes'
    # IEEE e4m3. Quantization is monotone, so the true argmax always
    # ties the row max and stays a candidate; map NaN to +inf so such
    # columns are candidates (rescoring decides) rather than poisoning
    # the row max.
    logits = np.where(np.isnan(logits), np.inf, logits)

    # Columns within DELTA of each row's max, rescored exactly in f64.
    m = logits.max(axis=1, keepdims=True)
    rows, cols = np.nonzero(logits >= m - DELTA)
    exact = np.einsum(
        "ij,ij->i", hs_sel[rows].astype(np.float64), W[cols].astype(np.float64)
    )
    ids = np.zeros(J, dtype=np.int64)
    best = np.full(J, -np.inf)
    for r, c, s in zip(rows, cols, exact):
        if s > best[r]:
            best[r] = s
            ids[r] = c
    return ids.astype(np.int32)
